# revision 5
# baseline (speedup 1.0000x reference)
"""Trainium2 Bass kernel for the ANFIS forward pass (8-core data-parallel).

Math: with L[b,f,m] = -0.5*((X[b,f]-mu[f,m])/sigma[f,m])^2,
  miAlloc[b,r] = prod_f exp(L[b,f,rules[r,f]])
  out[b] = (miAlloc @ c) / (sum_r miAlloc + 1e-10),  c = consequents.sum(1)

Factor the 8 features into two halves of 4. Each half has 81 possible
membership tuples, so miAlloc[b,r] = W1[b,rho1(r)] * W2[b,rho2(r)] where
  W1[b,t] = exp(sum_{f<4} a[f,tf]*(X[b,f]-mu[f,tf])^2),  a = -0.5/sigma^2
and rho1/rho2 map each rule to its half-tuple index. Then with
  C2[t1,t2] = sum_{r: rho(r)=(t1,t2)} c[r],   D2[t1,t2] = #{r: rho(r)=(t1,t2)}
(exact for arbitrary `rules`, duplicates included):
  num[b] = sum_{t2} (C2^T W1T)[t2,b] * W2T[t2,b]
  den[b] = sum_{t2} (D2^T W1T)[t2,b] * W2T[t2,b]
  out[b] = num[b] / (den[b] + 1e-10)

Per core (batch shard of 1024): two K=12 matmuls -> exp over [81,1024] ->
two K=81 matmuls -> elementwise product -> ones-reduce matmul -> divide.
"""

import numpy as np

import concourse.bass as bass
import concourse.tile as tile
from concourse import bacc, mybir
from concourse.bass_utils import run_bass_kernel_spmd

B, F, M = 8192, 8, 3
NC = 8
BC = B // NC  # 1024 batch rows per core
T = M**4  # 81 tuples per feature-half
FP32 = mybir.dt.float32
AF = mybir.ActivationFunctionType

_CACHE = {}


def _build_graph():
    nc = bacc.Bacc("TRN2", target_bir_lowering=False, debug=False, num_devices=NC)

    xt_ext = nc.dram_tensor("xt", [44, BC], FP32, kind="ExternalInput").ap()
    negmu_ext = nc.dram_tensor("negmu", [44, 1], FP32, kind="ExternalInput").ap()
    eb_ext = nc.dram_tensor("eb", [44, T], FP32, kind="ExternalInput").ap()
    c2_ext = nc.dram_tensor("c2", [T, T], FP32, kind="ExternalInput").ap()
    d2_ext = nc.dram_tensor("d2", [T, T], FP32, kind="ExternalInput").ap()
    ones_ext = nc.dram_tensor("ones", [T, 1], FP32, kind="ExternalInput").ap()
    out_ext = nc.dram_tensor("out", [1, BC], FP32, kind="ExternalOutput").ap()

    with tile.TileContext(nc) as tc:
        with (
            tc.tile_pool(name="const", bufs=1) as const,
            tc.tile_pool(name="work", bufs=1) as work,
            tc.tile_pool(name="psum", bufs=1, space=bass.MemorySpace.PSUM) as psum,
        ):
            xt = const.tile([44, BC], FP32)
            nc.sync.dma_start(out=xt[:, :], in_=xt_ext[:, :])
            negmu = const.tile([44, 1], FP32)
            nc.sync.dma_start(out=negmu[:, :], in_=negmu_ext[:, :])
            eb = const.tile([44, T], FP32)
            nc.sync.dma_start(out=eb[:, :], in_=eb_ext[:, :])
            c2 = const.tile([T, T], FP32)
            nc.sync.dma_start(out=c2[:, :], in_=c2_ext[:, :])
            d2 = const.tile([T, T], FP32)
            nc.sync.dma_start(out=d2[:, :], in_=d2_ext[:, :])
            ones = const.tile([T, 1], FP32)
            nc.sync.dma_start(out=ones[:, :], in_=ones_ext[:, :])

            # xc = X - mu (per-partition bias), sq = xc^2
            xc = work.tile([44, BC], FP32)
            nc.scalar.activation(xc[:, :], xt[:, :], AF.Identity, bias=negmu[:, :])
            sq = work.tile([44, BC], FP32)
            nc.vector.tensor_mul(sq[:, :], xc[:, :], xc[:, :])

            # logW1T/logW2T: [81, BC] = eb^T @ sq  (K=12 each, two PE row groups)
            lw1 = psum.tile([T, BC], FP32, tag="pa")
            lw2 = psum.tile([T, BC], FP32, tag="pb")
            for h in range(BC // 512):
                s = bass.ts(h, 512)
                nc.tensor.matmul(lw1[:, s], lhsT=eb[0:12, :], rhs=sq[0:12, s])
                nc.tensor.matmul(lw2[:, s], lhsT=eb[32:44, :], rhs=sq[32:44, s])

            w1 = work.tile([T, BC], FP32)
            nc.scalar.activation(w1[:, :], lw1[:, :], AF.Exp)
            w2 = work.tile([T, BC], FP32)
            nc.scalar.activation(w2[:, :], lw2[:, :], AF.Exp)

            # HT = C2^T @ W1T, HDT = D2^T @ W1T  (K=81)
            ht = psum.tile([T, BC], FP32, tag="pc")
            hd = psum.tile([T, BC], FP32, tag="pd")
            for h in range(BC // 512):
                s = bass.ts(h, 512)
                nc.tensor.matmul(ht[:, s], lhsT=c2[:, :], rhs=w1[:, s])
                nc.tensor.matmul(hd[:, s], lhsT=d2[:, :], rhs=w1[:, s])

            p1 = work.tile([T, BC], FP32)
            nc.vector.tensor_mul(p1[:, :], ht[:, :], w2[:, :])
            pd = work.tile([T, BC], FP32)
            nc.vector.tensor_mul(pd[:, :], hd[:, :], w2[:, :])

            # Partition-reduce over t2 via ones matmul -> [1, BC]
            nps = psum.tile([1, BC], FP32, tag="pa")
            dps = psum.tile([1, BC], FP32, tag="pb")
            for h in range(BC // 512):
                s = bass.ts(h, 512)
                nc.tensor.matmul(nps[:, s], lhsT=ones[:, :], rhs=p1[:, s])
                nc.tensor.matmul(dps[:, s], lhsT=ones[:, :], rhs=pd[:, s])

            dene = work.tile([1, BC], FP32)
            nc.vector.tensor_scalar_add(dene[:, :], dps[:, :], 1e-10)
            rden = work.tile([1, BC], FP32)
            nc.vector.reciprocal(rden[:, :], dene[:, :])
            outt = work.tile([1, BC], FP32)
            nc.vector.tensor_mul(outt[:, :], nps[:, :], rden[:, :])

            nc.sync.dma_start(out=out_ext[:, :], in_=outt[:, :])

    nc.compile()
    return nc


def _get_graph():
    if "nc" not in _CACHE:
        _CACHE["nc"] = _build_graph()
    return _CACHE["nc"]


def _prep_inputs(X, mu, sigma, consequents, rules):
    X = np.ascontiguousarray(np.asarray(X, dtype=np.float32))
    mu = np.asarray(mu, dtype=np.float32)
    sigma = np.asarray(sigma, dtype=np.float32)
    c = np.asarray(consequents, dtype=np.float32).sum(axis=1)
    r = np.asarray(rules).astype(np.int64)

    a = (-0.5 / (np.asarray(sigma, np.float64) ** 2)).astype(np.float32)  # [F, M]

    # tuple digit f of t (m0 major), t in [0, 81)
    digits = (np.arange(T)[:, None] // np.array([27, 9, 3, 1])[None, :]) % 3  # [81, 4]

    eb = np.zeros((44, T), np.float32)
    negmu = np.zeros((44, 1), np.float32)
    for f in range(4):
        for m in range(3):
            sel = (digits[:, f] == m).astype(np.float32)
            eb[3 * f + m, :] = a[f, m] * sel
            eb[32 + 3 * f + m, :] = a[4 + f, m] * sel
            negmu[3 * f + m, 0] = -mu[f, m]
            negmu[32 + 3 * f + m, 0] = -mu[4 + f, m]

    Xsh = X.reshape(NC, BC, F)
    xt = np.zeros((NC, 44, BC), np.float32)
    for f in range(4):
        for m in range(3):
            xt[:, 3 * f + m, :] = Xsh[:, :, f]
            xt[:, 32 + 3 * f + m, :] = Xsh[:, :, 4 + f]

    rho1 = ((r[:, 0] * 3 + r[:, 1]) * 3 + r[:, 2]) * 3 + r[:, 3]
    rho2 = ((r[:, 4] * 3 + r[:, 5]) * 3 + r[:, 6]) * 3 + r[:, 7]
    C2 = np.zeros((T, T), np.float64)
    np.add.at(C2, (rho1, rho2), c.astype(np.float64))
    D2 = np.zeros((T, T), np.float64)
    np.add.at(D2, (rho1, rho2), 1.0)

    shared = {
        "negmu": negmu,
        "eb": eb,
        "c2": np.ascontiguousarray(C2.astype(np.float32)),
        "d2": np.ascontiguousarray(D2.astype(np.float32)),
        "ones": np.ones((T, 1), np.float32),
    }
    in_maps = [
        {"xt": np.ascontiguousarray(xt[i]), **shared} for i in range(NC)
    ]
    return in_maps


def _run(in_maps, trace=False, **kwargs):
    nc = _get_graph()
    return run_bass_kernel_spmd(
        nc, in_maps, core_ids=list(range(NC)), trace=trace, **kwargs
    )


def kernel(X, mu, sigma, consequents, rules):
    in_maps = _prep_inputs(X, mu, sigma, consequents, rules)
    res = _run(in_maps)
    out = np.concatenate(
        [np.asarray(res.results[i]["out"]).reshape(BC) for i in range(NC)]
    )
    return out.astype(np.float32)


# revision 7
# speedup vs baseline: 1.6300x; 1.6300x over previous
"""Trainium2 Bass kernel for the ANFIS forward pass (8-core data-parallel).

Math: with L[b,f,m] = -0.5*((X[b,f]-mu[f,m])/sigma[f,m])^2,
  miAlloc[b,r] = prod_f exp(L[b,f,rules[r,f]])
  out[b] = (miAlloc @ c) / (sum_r miAlloc + 1e-10),  c = consequents.sum(1)

Factor the 8 features into two halves of 4. Each half has 81 possible
membership tuples, so miAlloc[b,r] = W1[b,rho1(r)] * W2[b,rho2(r)] where
  W1[b,t] = exp(sum_{f<4} a[f,tf]*(X[b,f]-mu[f,tf])^2),  a = -0.5/sigma^2
and rho1/rho2 map each rule to its half-tuple index. Then with
  C2[t1,t2] = sum_{r: rho(r)=(t1,t2)} c[r],   D2[t1,t2] = #{r: rho(r)=(t1,t2)}
(exact for arbitrary `rules`, duplicates included):
  num[b] = sum_{t2} (C2^T W1T)[t2,b] * W2T[t2,b]
  den[b] = sum_{t2} (D2^T W1T)[t2,b] * W2T[t2,b]
  out[b] = num[b] / (den[b] + 1e-10)

Per core (batch shard of 1024): one Square activation, two K=12 bf16
matmuls, exp over [81,1024], two K=81 bf16 matmuls, elementwise product,
ones-reduce matmul, then 1/(den+eps) via exp(-ln(den+eps)) on ScalarE.
"""

import numpy as np
import ml_dtypes

import concourse.bass as bass
import concourse.tile as tile
from concourse import bacc, mybir
from concourse.bass_utils import run_bass_kernel_spmd

B, F, M = 8192, 8, 3
NC = 8
BC = B // NC  # 1024 batch rows per core
T = M**4  # 81 tuples per feature-half
FP32 = mybir.dt.float32
BF16 = mybir.dt.bfloat16
AF = mybir.ActivationFunctionType
NP_BF16 = ml_dtypes.bfloat16

_CACHE = {}


def _build_graph():
    nc = bacc.Bacc("TRN2", target_bir_lowering=False, debug=False, num_devices=NC)

    xt_ext = nc.dram_tensor("xt", [44, BC], FP32, kind="ExternalInput").ap()
    negmu_ext = nc.dram_tensor("negmu", [44, 1], FP32, kind="ExternalInput").ap()
    eb_ext = nc.dram_tensor("eb", [44, T], BF16, kind="ExternalInput").ap()
    c2_ext = nc.dram_tensor("c2", [T, T], BF16, kind="ExternalInput").ap()
    d2_ext = nc.dram_tensor("d2", [T, T], BF16, kind="ExternalInput").ap()
    ones_ext = nc.dram_tensor("ones", [T, 1], BF16, kind="ExternalInput").ap()
    out_ext = nc.dram_tensor("out", [1, BC], FP32, kind="ExternalOutput").ap()

    with tile.TileContext(nc) as tc:
        with (
            tc.tile_pool(name="const", bufs=1) as const,
            tc.tile_pool(name="work", bufs=1) as work,
            tc.tile_pool(name="psum", bufs=1, space=bass.MemorySpace.PSUM) as psum,
        ):
            # spread input DMAs across engine queues; xt is on the critical path
            xt = const.tile([44, BC], FP32)
            nc.sync.dma_start(out=xt[:, :], in_=xt_ext[:, :])
            negmu = const.tile([44, 1], FP32)
            nc.scalar.dma_start(out=negmu[:, :], in_=negmu_ext[:, :])
            eb = const.tile([44, T], BF16)
            nc.gpsimd.dma_start(out=eb[:, :], in_=eb_ext[:, :])
            c2 = const.tile([T, T], BF16)
            nc.scalar.dma_start(out=c2[:, :], in_=c2_ext[:, :])
            d2 = const.tile([T, T], BF16)
            nc.sync.dma_start(out=d2[:, :], in_=d2_ext[:, :])
            ones = const.tile([T, 1], BF16)
            nc.gpsimd.dma_start(out=ones[:, :], in_=ones_ext[:, :])
            epsb = const.tile([1, 1], FP32)
            nc.vector.memset(epsb[:, :], 1e-10)

            # sq = (x - mu)^2, cast to bf16 for the matmul
            sq = work.tile([44, BC], BF16)
            nc.scalar.activation(sq[:, :], xt[:, :], AF.Square, bias=negmu[:, :])

            # logW1T/logW2T: [81, BC] = eb^T @ sq  (K=12 each, two PE row groups)
            lw1 = psum.tile([T, BC], FP32, tag="pa")
            lw2 = psum.tile([T, BC], FP32, tag="pb")
            for h in range(BC // 512):
                s = bass.ts(h, 512)
                nc.tensor.matmul(lw1[:, s], lhsT=eb[0:12, :], rhs=sq[0:12, s])
                nc.tensor.matmul(lw2[:, s], lhsT=eb[32:44, :], rhs=sq[32:44, s])

            w1 = work.tile([T, BC], BF16)
            nc.scalar.activation(w1[:, :], lw1[:, :], AF.Exp)
            w2 = work.tile([T, BC], BF16)
            nc.scalar.activation(w2[:, :], lw2[:, :], AF.Exp)

            # HT = C2^T @ W1T, HDT = D2^T @ W1T  (K=81)
            ht = psum.tile([T, BC], FP32, tag="pc")
            hd = psum.tile([T, BC], FP32, tag="pd")
            for h in range(BC // 512):
                s = bass.ts(h, 512)
                nc.tensor.matmul(ht[:, s], lhsT=c2[:, :], rhs=w1[:, s])
                nc.tensor.matmul(hd[:, s], lhsT=d2[:, :], rhs=w1[:, s])

            p1 = work.tile([T, BC], BF16)
            nc.vector.tensor_mul(p1[:, :], ht[:, :], w2[:, :])
            pd = work.tile([T, BC], BF16)
            nc.vector.tensor_mul(pd[:, :], hd[:, :], w2[:, :])

            # Partition-reduce over t2 via ones matmul -> [1, BC]
            nps = psum.tile([1, BC], FP32, tag="pa")
            dps = psum.tile([1, BC], FP32, tag="pb")
            for h in range(BC // 512):
                s = bass.ts(h, 512)
                nc.tensor.matmul(nps[:, s], lhsT=ones[:, :], rhs=p1[:, s])
                nc.tensor.matmul(dps[:, s], lhsT=ones[:, :], rhs=pd[:, s])

            # 1/(den+eps) = exp(-ln(den+eps)) on ScalarE (same ACT table set)
            lden = work.tile([1, BC], FP32)
            nc.scalar.activation(lden[:, :], dps[:, :], AF.Ln, bias=epsb[:, :])
            rden = work.tile([1, BC], FP32)
            nc.scalar.activation(rden[:, :], lden[:, :], AF.Exp, scale=-1.0)
            outt = work.tile([1, BC], FP32)
            nc.vector.tensor_mul(outt[:, :], nps[:, :], rden[:, :])

            nc.sync.dma_start(out=out_ext[:, :], in_=outt[:, :])

    nc.compile()
    return nc


def _get_graph():
    if "nc" not in _CACHE:
        _CACHE["nc"] = _build_graph()
    return _CACHE["nc"]


def _prep_inputs(X, mu, sigma, consequents, rules):
    X = np.ascontiguousarray(np.asarray(X, dtype=np.float32))
    mu = np.asarray(mu, dtype=np.float32)
    sigma = np.asarray(sigma, dtype=np.float32)
    c = np.asarray(consequents, dtype=np.float32).sum(axis=1)
    r = np.asarray(rules).astype(np.int64)

    a = (-0.5 / (np.asarray(sigma, np.float64) ** 2)).astype(np.float32)  # [F, M]

    # tuple digit f of t (m0 major), t in [0, 81)
    digits = (np.arange(T)[:, None] // np.array([27, 9, 3, 1])[None, :]) % 3  # [81, 4]

    eb = np.zeros((44, T), np.float32)
    negmu = np.zeros((44, 1), np.float32)
    for f in range(4):
        for m in range(3):
            sel = (digits[:, f] == m).astype(np.float32)
            eb[3 * f + m, :] = a[f, m] * sel
            eb[32 + 3 * f + m, :] = a[4 + f, m] * sel
            negmu[3 * f + m, 0] = -mu[f, m]
            negmu[32 + 3 * f + m, 0] = -mu[4 + f, m]

    Xsh = X.reshape(NC, BC, F)
    xt = np.zeros((NC, 44, BC), np.float32)
    for f in range(4):
        for m in range(3):
            xt[:, 3 * f + m, :] = Xsh[:, :, f]
            xt[:, 32 + 3 * f + m, :] = Xsh[:, :, 4 + f]

    rho1 = ((r[:, 0] * 3 + r[:, 1]) * 3 + r[:, 2]) * 3 + r[:, 3]
    rho2 = ((r[:, 4] * 3 + r[:, 5]) * 3 + r[:, 6]) * 3 + r[:, 7]
    C2 = np.zeros((T, T), np.float64)
    np.add.at(C2, (rho1, rho2), c.astype(np.float64))
    D2 = np.zeros((T, T), np.float64)
    np.add.at(D2, (rho1, rho2), 1.0)

    shared = {
        "negmu": negmu,
        "eb": np.ascontiguousarray(eb.astype(NP_BF16)),
        "c2": np.ascontiguousarray(C2.astype(np.float32).astype(NP_BF16)),
        "d2": np.ascontiguousarray(D2.astype(np.float32).astype(NP_BF16)),
        "ones": np.ones((T, 1), NP_BF16),
    }
    in_maps = [
        {"xt": np.ascontiguousarray(xt[i]), **shared} for i in range(NC)
    ]
    return in_maps


def _run(in_maps, trace=False, **kwargs):
    nc = _get_graph()
    return run_bass_kernel_spmd(
        nc, in_maps, core_ids=list(range(NC)), trace=trace, **kwargs
    )


def kernel(X, mu, sigma, consequents, rules):
    in_maps = _prep_inputs(X, mu, sigma, consequents, rules)
    res = _run(in_maps)
    out = np.concatenate(
        [np.asarray(res.results[i]["out"]).reshape(BC) for i in range(NC)]
    )
    return out.astype(np.float32)


# revision 10
# speedup vs baseline: 1.6639x; 1.0208x over previous
"""Trainium2 Bass kernel for the ANFIS forward pass (8-core data-parallel).

Math: with L[b,f,m] = -0.5*((X[b,f]-mu[f,m])/sigma[f,m])^2,
  miAlloc[b,r] = prod_f exp(L[b,f,rules[r,f]])
  out[b] = (miAlloc @ c) / (sum_r miAlloc + 1e-10),  c = consequents.sum(1)

Factor the 8 features into two halves of 4. Each half has 81 possible
membership tuples, so miAlloc[b,r] = W1[b,rho1(r)] * W2[b,rho2(r)] where
  W1[b,t] = exp(sum_{f<4} a[f,tf]*(X[b,f]-mu[f,tf])^2),  a = -0.5/sigma^2
and rho1/rho2 map each rule to its half-tuple index. Then with
  C2[t1,t2] = sum_{r: rho(r)=(t1,t2)} c[r],   D2[t1,t2] = #{r: rho(r)=(t1,t2)}
(exact for arbitrary `rules`, duplicates included):
  num[b] = sum_{t2} (C2^T W1T)[t2,b] * W2T[t2,b]
  den[b] = sum_{t2} (D2^T W1T)[t2,b] * W2T[t2,b]
  out[b] = num[b] / (den[b] + 1e-10)

Per core (batch shard of 1024): one Square activation, two K=12 bf16
matmuls, exp over [81,1024], two K=81 bf16 matmuls, elementwise product,
ones-reduce matmul, then 1/(den+eps) via exp(-ln(den+eps)) on ScalarE.
"""

import numpy as np
import ml_dtypes

import concourse.bass as bass
import concourse.tile as tile
from concourse import bacc, mybir
from concourse.bass_utils import run_bass_kernel_spmd

B, F, M = 8192, 8, 3
NC = 8
BC = B // NC  # 1024 batch rows per core
T = M**4  # 81 tuples per feature-half
FP32 = mybir.dt.float32
BF16 = mybir.dt.bfloat16
AF = mybir.ActivationFunctionType
NP_BF16 = ml_dtypes.bfloat16

_CACHE = {}


def _build_graph():
    nc = bacc.Bacc("TRN2", target_bir_lowering=False, debug=False, num_devices=NC)

    # xt: col 0 = -mu (per-partition bias), cols 1.. = X^T replicated rows
    xt_ext = nc.dram_tensor("xt", [44, 1 + BC], FP32, kind="ExternalInput").ap()
    # bigc: [81, 244] bf16 = C2 | D2 | ones | eb (eb on rows 0:44)
    bigc_ext = nc.dram_tensor("bigc", [T, 3 * T + 1], BF16, kind="ExternalInput").ap()
    out_ext = nc.dram_tensor("out", [1, BC], FP32, kind="ExternalOutput").ap()

    with tile.TileContext(nc) as tc:
        with (
            tc.tile_pool(name="const", bufs=1) as const,
            tc.tile_pool(name="work", bufs=1) as work,
            tc.tile_pool(name="psum", bufs=1, space=bass.MemorySpace.PSUM) as psum,
        ):
            xt = const.tile([44, 1 + BC], FP32)
            nc.sync.dma_start(out=xt[:, :], in_=xt_ext[:, :])
            bigc = const.tile([T, 3 * T + 1], BF16)
            nc.scalar.dma_start(out=bigc[:, :], in_=bigc_ext[:, :])
            c2 = bigc[:, 0:T]
            d2 = bigc[:, T : 2 * T]
            ones = bigc[:, 2 * T : 2 * T + 1]
            eb_q0 = bigc[0:12, 2 * T + 1 : 3 * T + 1]
            eb_q32 = bigc[32:44, 2 * T + 1 : 3 * T + 1]
            epsb = const.tile([1, 1], FP32)
            nc.vector.memset(epsb[:, :], 1e-10)

            # sq = (x - mu)^2, cast to bf16 for the matmul
            sq = work.tile([44, BC], BF16)
            nc.scalar.activation(
                sq[:, :], xt[:, 1 : 1 + BC], AF.Square, bias=xt[:, 0:1]
            )

            # logW: [81, 2048] = [logW1T halves | logW2T halves] (K=12, 2 row grps)
            lw = psum.tile([T, 2 * BC], FP32, tag="pa")
            for h in range(BC // 512):
                s = bass.ts(h, 512)
                nc.tensor.matmul(lw[:, s], lhsT=eb_q0, rhs=sq[0:12, s])
                nc.tensor.matmul(
                    lw[:, bass.ds(BC + h * 512, 512)], lhsT=eb_q32, rhs=sq[32:44, s]
                )

            w = work.tile([T, 2 * BC], BF16)
            nc.scalar.activation(w[:, :], lw[:, :], AF.Exp)
            w1 = w[:, 0:BC]
            w2 = w[:, BC : 2 * BC]

            # HT = C2^T @ W1T, HDT = D2^T @ W1T  (K=81)
            ht = psum.tile([T, BC], FP32, tag="pc")
            hd = psum.tile([T, BC], FP32, tag="pd")
            for h in range(BC // 512):
                s = bass.ts(h, 512)
                nc.tensor.matmul(ht[:, s], lhsT=c2, rhs=w1[:, s])
                nc.tensor.matmul(hd[:, s], lhsT=d2, rhs=w1[:, s])

            p1 = work.tile([T, BC], BF16)
            nc.vector.tensor_mul(p1[:, :], ht[:, :], w2)
            pd = work.tile([T, BC], BF16)
            nc.vector.tensor_mul(pd[:, :], hd[:, :], w2)

            # Partition-reduce over t2 via ones matmul -> [1, BC]
            nps = psum.tile([1, BC], FP32, tag="pc")
            dps = psum.tile([1, BC], FP32, tag="pd")
            for h in range(BC // 512):
                s = bass.ts(h, 512)
                nc.tensor.matmul(nps[:, s], lhsT=ones, rhs=p1[:, s])
                nc.tensor.matmul(dps[:, s], lhsT=ones, rhs=pd[:, s])

            # out = num * 1/(den+eps); approx recip is ~51 ULP, plenty for 2e-2
            dene = work.tile([1, BC], FP32)
            nc.scalar.activation(dene[:, :], dps[:, :], AF.Identity, bias=epsb[:, :])
            rden = work.tile([1, BC], FP32)
            nc.vector.reciprocal_approx_fast(rden[:, :], dene[:, :])
            outt = work.tile([1, BC], FP32)
            nc.vector.tensor_mul(outt[:, :], nps[:, :], rden[:, :])

            nc.sync.dma_start(out=out_ext[:, :], in_=outt[:, :])

    nc.compile()
    return nc


def _get_graph():
    if "nc" not in _CACHE:
        _CACHE["nc"] = _build_graph()
    return _CACHE["nc"]


def _prep_inputs(X, mu, sigma, consequents, rules):
    X = np.ascontiguousarray(np.asarray(X, dtype=np.float32))
    mu = np.asarray(mu, dtype=np.float32)
    sigma = np.asarray(sigma, dtype=np.float32)
    c = np.asarray(consequents, dtype=np.float32).sum(axis=1)
    r = np.asarray(rules).astype(np.int64)

    a = (-0.5 / (np.asarray(sigma, np.float64) ** 2)).astype(np.float32)  # [F, M]

    # tuple digit f of t (m0 major), t in [0, 81)
    digits = (np.arange(T)[:, None] // np.array([27, 9, 3, 1])[None, :]) % 3  # [81, 4]

    eb = np.zeros((44, T), np.float32)
    negmu = np.zeros((44, 1), np.float32)
    for f in range(4):
        for m in range(3):
            sel = (digits[:, f] == m).astype(np.float32)
            eb[3 * f + m, :] = a[f, m] * sel
            eb[32 + 3 * f + m, :] = a[4 + f, m] * sel
            negmu[3 * f + m, 0] = -mu[f, m]
            negmu[32 + 3 * f + m, 0] = -mu[4 + f, m]

    Xsh = X.reshape(NC, BC, F)
    xt = np.zeros((NC, 44, 1 + BC), np.float32)
    xt[:, :, 0] = negmu[None, :, 0]
    for f in range(4):
        for m in range(3):
            xt[:, 3 * f + m, 1:] = Xsh[:, :, f]
            xt[:, 32 + 3 * f + m, 1:] = Xsh[:, :, 4 + f]

    rho1 = ((r[:, 0] * 3 + r[:, 1]) * 3 + r[:, 2]) * 3 + r[:, 3]
    rho2 = ((r[:, 4] * 3 + r[:, 5]) * 3 + r[:, 6]) * 3 + r[:, 7]
    C2 = np.zeros((T, T), np.float64)
    np.add.at(C2, (rho1, rho2), c.astype(np.float64))
    D2 = np.zeros((T, T), np.float64)
    np.add.at(D2, (rho1, rho2), 1.0)

    bigc = np.zeros((T, 3 * T + 1), np.float32)
    bigc[:, 0:T] = C2.astype(np.float32)
    bigc[:, T : 2 * T] = D2.astype(np.float32)
    bigc[:, 2 * T] = 1.0
    bigc[0:44, 2 * T + 1 :] = eb
    bigc = np.ascontiguousarray(bigc.astype(NP_BF16))

    in_maps = [
        {"xt": np.ascontiguousarray(xt[i]), "bigc": bigc} for i in range(NC)
    ]
    return in_maps


def _run(in_maps, trace=False, **kwargs):
    nc = _get_graph()
    return run_bass_kernel_spmd(
        nc, in_maps, core_ids=list(range(NC)), trace=trace, **kwargs
    )


def kernel(X, mu, sigma, consequents, rules):
    in_maps = _prep_inputs(X, mu, sigma, consequents, rules)
    res = _run(in_maps)
    out = np.concatenate(
        [np.asarray(res.results[i]["out"]).reshape(BC) for i in range(NC)]
    )
    return out.astype(np.float32)
